# revision 10
# baseline (speedup 1.0000x reference)
"""Trainium2 Bass kernel for CoupledFlowMatching (32-step Euler flow + Hutchinson
log-det via forward-mode JVP).

Math (per batch row n, per step s, t_mid = (s+0.5)/32):
    x    = [a, s_ctx, t]                           (81 feats; t folded into bias)
    pre1 = x @ W1 + b1          h1 = relu(pre1)
    pre2 = h1 @ W2 + b2         h2 = relu(pre2)
    v    = h2 @ W3v + b3v       (W3v = W3[:, :16] + W3[:, 16:], expert fold)
    JVP with tangent da = rad (+-1):
    dpre1 = rad @ W1a ; dh1 = relu'(pre1+b1) * dpre1
    dpre2 = dh1 @ W2  ; dh2 = relu'(pre2+b2) * dpre2
    dv    = dh2 @ W3v
    div   = sum_f dv*rad ;  logdet -= DELTA*div ;  a += DELTA*v
Final: logp = sum_f(-.5 a^2 - .5 log2pi) + logdet.

Layout: everything feature-major [feat, batch] so weights are the stationary
matmul operand (lhsT) verbatim and the JVP reuses the same (untransposed)
weights as the forward. rad is pre-scaled by -DELTA on the host: the JVP is
linear so dv' = -DELTA*dv, and sum(dv'*rad') = DELTA^2 * sum(dv*rad); the
final reduction weight -1/DELTA recovers logdet exactly.

All matmul operands are float32r (TF32-class, 1 PE cycle/row vs 4 for fp32);
walrus requires every producer feeding an f32r matmul to emit f32r.
relu' masks come from h (SBUF) because DVE has a single PSUM read port.

Sharding: pure data parallel, batch 65536 -> 8 cores x 8192.
"""

import os
import sys

if "/opt/trn_rl_repo" not in sys.path:
    sys.path.insert(0, "/opt/trn_rl_repo")

import numpy as np

import concourse.bacc as bacc
import concourse.bass as bass
import concourse.tile as tile
from concourse import mybir
from concourse.bass_utils import run_bass_kernel_spmd

N_STEPS = 32
DELTA = 1.0 / N_STEPS
LOG_2PI = float(np.log(2.0 * np.pi))
N_CORES = 8
TILE = 512

F32 = mybir.dt.float32
F32R = mybir.dt.float32r
AF = mybir.ActivationFunctionType
OP = mybir.AluOpType

last_exec_time_ns = None
last_results = None


def build_kernel(b_shard: int, has_b2: bool, has_b3v: bool) -> bass.Bass:
    nc = bacc.Bacc(trn_type="TRN2", target_bir_lowering=False)
    ntiles = b_shard // TILE
    assert b_shard % TILE == 0

    sT = nc.dram_tensor("sT", [64, b_shard], F32R, kind="ExternalInput")
    a0T = nc.dram_tensor("a0T", [16, b_shard], F32R, kind="ExternalInput")
    radT = nc.dram_tensor("radT", [N_STEPS, 16, b_shard], F32R, kind="ExternalInput")
    W1d = nc.dram_tensor("W1d", [80, 128], F32R, kind="ExternalInput")
    W1ad = nc.dram_tensor("W1ad", [16, 128], F32R, kind="ExternalInput")
    W2d = nc.dram_tensor("W2d", [128, 128], F32R, kind="ExternalInput")
    W3d = nc.dram_tensor("W3d", [128, 32], F32R, kind="ExternalInput")
    b1td = nc.dram_tensor("b1td", [128, N_STEPS], F32, kind="ExternalInput")
    b2d = nc.dram_tensor("b2d", [128, 1], F32, kind="ExternalInput")
    b3vd = nc.dram_tensor("b3vd", [16, 1], F32, kind="ExternalInput")
    rwd = nc.dram_tensor("rwd", [16, 2], F32R, kind="ExternalInput")
    fbd = nc.dram_tensor("fbd", [1, 1], F32, kind="ExternalInput")
    outd = nc.dram_tensor("out", [1, b_shard], F32, kind="ExternalOutput")

    with tile.TileContext(nc) as tc:
        with (
            tc.tile_pool(name="const", bufs=1) as cpool,
            tc.tile_pool(name="work", bufs=2) as wpool,
            tc.tile_pool(name="rstream", bufs=2) as rpool,
            tc.tile_pool(name="ps", bufs=1, space=bass.MemorySpace.PSUM) as psum,
        ):
            W1s = cpool.tile([80, 128], F32R)
            nc.sync.dma_start(W1s[:], W1d[:])
            W1as = cpool.tile([16, 128], F32R)
            nc.sync.dma_start(W1as[:], W1ad[:])
            W2s = cpool.tile([128, 128], F32R)
            nc.sync.dma_start(W2s[:], W2d[:])
            W3s = cpool.tile([128, 32], F32R)
            nc.sync.dma_start(W3s[:], W3d[:])
            b1ts = cpool.tile([128, N_STEPS], F32)
            nc.sync.dma_start(b1ts[:], b1td[:])
            b2s = cpool.tile([128, 1], F32)
            nc.sync.dma_start(b2s[:], b2d[:])
            b3vs = cpool.tile([16, 1], F32)
            nc.sync.dma_start(b3vs[:], b3vd[:])
            rws = cpool.tile([16, 2], F32R)
            nc.sync.dma_start(rws[:], rwd[:])

            xT = cpool.tile([80, b_shard], F32R)
            nc.sync.dma_start(xT[0:16, :], a0T[:])
            nc.sync.dma_start(xT[16:80, :], sT[:])
            accT = cpool.tile([16, b_shard], F32R)
            fout = cpool.tile([1, b_shard], F32)
            fbias = cpool.tile([1, 1], F32)
            nc.sync.dma_start(fbias[:], fbd[:])

            for s in range(N_STEPS):
                radsb = rpool.tile([16, b_shard], F32R, tag="radsb")
                nc.sync.dma_start(radsb[:], radT[s])
                b1c = b1ts[:, s : s + 1]
                for t in range(ntiles):
                    cols = slice(t * TILE, (t + 1) * TILE)
                    pre1 = psum.tile([128, TILE], F32, tag="pre1")
                    dpre1 = psum.tile([128, TILE], F32, tag="dpre1")
                    nc.tensor.matmul(
                        pre1[:], W1s[:], xT[:, cols], start=True, stop=True
                    )
                    nc.tensor.matmul(
                        dpre1[:], W1as[:], radsb[:, cols], start=True, stop=True
                    )
                    h1 = wpool.tile([128, TILE], F32R, tag="h1")
                    nc.scalar.activation(h1[:], pre1[:], AF.Relu, bias=b1c)
                    dh1 = wpool.tile([128, TILE], F32R, tag="dh1")
                    nc.vector.scalar_tensor_tensor(
                        dh1[:], h1[:], 0.0, dpre1[:], OP.is_gt, OP.mult
                    )
                    pre2 = psum.tile([128, TILE], F32, tag="pre2")
                    dpre2 = psum.tile([128, TILE], F32, tag="dpre2")
                    nc.tensor.matmul(
                        pre2[:], W2s[:], h1[:], start=True, stop=True
                    )
                    nc.tensor.matmul(
                        dpre2[:], W2s[:], dh1[:], start=True, stop=True
                    )
                    h2 = wpool.tile([128, TILE], F32R, tag="h2")
                    if has_b2:
                        nc.scalar.activation(h2[:], pre2[:], AF.Relu, bias=b2s[:, 0:1])
                    else:
                        nc.scalar.activation(h2[:], pre2[:], AF.Relu)
                    dh2 = wpool.tile([128, TILE], F32R, tag="dh2")
                    nc.vector.scalar_tensor_tensor(
                        dh2[:], h2[:], 0.0, dpre2[:], OP.is_gt, OP.mult
                    )
                    vps = psum.tile([32, TILE], F32, tag="vps")
                    dvps = psum.tile([32, TILE], F32, tag="dvps")
                    nc.tensor.matmul(
                        vps[:], W3s[:], h2[:], start=True, stop=True
                    )
                    nc.tensor.matmul(
                        dvps[:], W3s[:], dh2[:], start=True, stop=True
                    )
                    # Euler update, in place on the a-rows of xT
                    nc.vector.scalar_tensor_tensor(
                        xT[0:16, cols], vps[0:16, :], DELTA, xT[0:16, cols],
                        OP.mult, OP.add,
                    )
                    if has_b3v:
                        nc.vector.tensor_scalar_add(
                            xT[0:16, cols], xT[0:16, cols], b3vs[:, 0:1]
                        )
                    if s == 0:
                        nc.vector.tensor_mul(
                            accT[:, cols], dvps[0:16, :], radsb[:, cols]
                        )
                    else:
                        prod = wpool.tile([16, TILE], F32R, tag="prod")
                        nc.vector.tensor_mul(prod[:], dvps[0:16, :], radsb[:, cols])
                        nc.vector.tensor_add(accT[:, cols], accT[:, cols], prod[:])

            for t in range(ntiles):
                cols = slice(t * TILE, (t + 1) * TILE)
                z = wpool.tile([16, TILE], F32R, tag="z")
                nc.vector.tensor_mul(z[:], xT[0:16, cols], xT[0:16, cols])
                fin = psum.tile([1, TILE], F32, tag="fin")
                nc.tensor.matmul(
                    fin[:], rws[:, 0:1], z[:], start=True, stop=False
                )
                nc.tensor.matmul(
                    fin[:], rws[:, 1:2], accT[:, cols], start=False, stop=True
                )
                nc.scalar.activation(
                    fout[:, cols], fin[:], AF.Identity, bias=fbias[:, 0:1]
                )
            nc.sync.dma_start(outd[:], fout[:])

    nc.finalize()
    return nc


def make_in_maps(s, a0, rad, W1, b1, W2, b2, W3, b3, n_cores=N_CORES):
    """Host-side prep: shard along batch, transpose to feature-major, fold
    the t column / biases, pre-scale rad by -DELTA."""
    B = s.shape[0]
    b_shard = B // n_cores
    s = np.asarray(s, np.float32)
    a0 = np.asarray(a0, np.float32)
    W1 = np.asarray(W1, np.float32)
    b1 = np.asarray(b1, np.float32)
    W2 = np.asarray(W2, np.float32)
    b2 = np.asarray(b2, np.float32)
    W3 = np.asarray(W3, np.float32)
    b3 = np.asarray(b3, np.float32)

    t_mids = (np.arange(N_STEPS, dtype=np.float64) + 0.5) / N_STEPS
    t_mids = t_mids.astype(np.float32)
    # b1 + t_mid * W1[80, :], laid out [128 feat, 32 steps]
    b1t = (b1[None, :] + t_mids[:, None] * W1[80][None, :]).astype(np.float32).T
    b1t = np.ascontiguousarray(b1t)
    W1_80 = np.ascontiguousarray(W1[:80])
    W1a = np.ascontiguousarray(W1[:16])
    W3v = W3[:, :16] + W3[:, 16:]
    W3vp = np.concatenate([W3v, np.zeros((128, 16), np.float32)], axis=1)
    W3vp = np.ascontiguousarray(W3vp)
    b3v = (b3[:16] + b3[16:]).astype(np.float32)
    b2c = np.ascontiguousarray(b2[:, None])
    b3vc = np.ascontiguousarray((DELTA * b3v)[:, None])
    rw = np.stack(
        [np.full(16, -0.5, np.float32), np.full(16, -1.0 / DELTA, np.float32)], axis=1
    )
    rw = np.ascontiguousarray(rw)
    fb = np.full((1, 1), -8.0 * LOG_2PI, np.float32)

    has_b2 = bool(np.any(b2 != 0))
    has_b3v = bool(np.any(b3v != 0))

    in_maps = []
    for i in range(n_cores):
        sl = slice(i * b_shard, (i + 1) * b_shard)
        radf = (-DELTA) * (
            2.0 * np.asarray(rad[:, sl, :], np.float32) - 1.0
        )  # [32, b, 16]
        in_maps.append(
            {
                "sT": np.ascontiguousarray(s[sl].T),
                "a0T": np.ascontiguousarray(a0[sl].T),
                "radT": np.ascontiguousarray(radf.transpose(0, 2, 1)),
                "W1d": W1_80,
                "W1ad": W1a,
                "W2d": W2,
                "W3d": W3vp,
                "b1td": b1t,
                "b2d": b2c,
                "b3vd": b3vc,
                "rwd": rw,
                "fbd": fb,
            }
        )
    return in_maps, has_b2, has_b3v, b_shard


def kernel(**inputs) -> np.ndarray:
    global last_exec_time_ns, last_results
    in_maps, has_b2, has_b3v, b_shard = make_in_maps(**inputs)
    nc = build_kernel(b_shard, has_b2, has_b3v)
    trace = bool(os.environ.get("KERNEL_TRACE"))
    res = run_bass_kernel_spmd(
        nc, in_maps, core_ids=list(range(N_CORES)), trace=trace
    )
    last_exec_time_ns = res.exec_time_ns
    last_results = res
    out = np.concatenate([r["out"].reshape(-1) for r in res.results])
    return out.astype(np.float32)


# revision 31
# speedup vs baseline: 2.6419x; 2.6419x over previous
"""Trainium2 Bass kernel for CoupledFlowMatching (32-step Euler flow + Hutchinson
log-det via forward-mode JVP).

Math (per batch row n, per step s, t_mid = (s+0.5)/32):
    x    = [a, s_ctx, t]                           (81 feats; t folded into bias)
    pre1 = x @ W1 + b1          h1 = relu(pre1)
    pre2 = h1 @ W2 + b2         h2 = relu(pre2)
    v    = h2 @ W3v + b3v       (W3v = W3[:, :16] + W3[:, 16:], expert fold)
    JVP with tangent da = rad (+-1):
    dpre1 = rad @ W1a ; dh1 = relu'(pre1+b1) * dpre1
    dpre2 = dh1 @ W2  ; dh2 = relu'(pre2+b2) * dpre2
    dv    = dh2 @ W3v
    div   = sum_f dv*rad ;  logdet -= DELTA*div ;  a += DELTA*v
Final: logp = sum_f(-.5 a^2 - .5 log2pi) + logdet.

Layout: feature-major [feat, batch]; weights are the stationary matmul operand
(lhsT) verbatim; the JVP reuses the same untransposed weights as the forward.
rad is pre-scaled by -DELTA (JVP is linear: dv' = -DELTA*dv, and
sum(dv'*rad') = DELTA^2 sum(dv*rad); the -1/DELTA reduction weight recovers
logdet). All matmul inputs are float32r (1 PE cycle/row vs 4 for fp32).

Perf structure: batch tiles of 512 processed in groups of 4. Layer-3 outputs
v/dv for the 4 tiles of a group are FOLDED into one [128, 512] PSUM bank
(partitions 32j+f hold tile j's feature f) by giving tile j's L3/J3 matmul a
[128,128] lhsT with W3v at columns 32j..32j+15 and zeros elsewhere, all four
accumulating into the same bank. The 16-feature elementwise work (Euler
update, dv*rad, logdet accumulate) then runs at full 128-lane DVE width on the
folded tensors; the logdet accumulate runs on GpSimd from SBUF. The folded
a-state is DMA'd back to the flat a-rows of xT once per step.

Sharding: pure data parallel, batch 65536 -> 8 cores x 8192.
"""

import os
import sys

if "/opt/trn_rl_repo" not in sys.path:
    sys.path.insert(0, "/opt/trn_rl_repo")

import numpy as np

import concourse.bacc as bacc
import concourse.bass as bass
import concourse.tile as tile
from concourse import mybir
from concourse.bass_utils import run_bass_kernel_spmd

N_STEPS = 32
DELTA = 1.0 / N_STEPS
LOG_2PI = float(np.log(2.0 * np.pi))
N_CORES = 8
TILE = 512
GRP = 4  # batch tiles per fold group

F32 = mybir.dt.float32
F32R = mybir.dt.float32r
AF = mybir.ActivationFunctionType
OP = mybir.AluOpType

last_exec_time_ns = None
last_results = None


def build_kernel(b_shard: int, has_b2: bool, has_b3v: bool) -> bass.Bass:
    # dev-only A/B switches (default off => production path)
    no_adma = bool(os.environ.get("KV2_NO_ADMA"))
    acc_dve = bool(os.environ.get("KV2_ACC_DVE"))
    no_l3 = bool(os.environ.get("KV2_NO_L3"))
    no_msel = bool(os.environ.get("KV2_NO_MSEL"))
    no_relu = bool(os.environ.get("KV2_NO_RELU"))
    nc = bacc.Bacc(trn_type="TRN2", target_bir_lowering=False)
    ntiles = b_shard // TILE
    ngroups = ntiles // GRP
    assert b_shard % (TILE * GRP) == 0
    gw = GRP * TILE  # columns per group in folded tensors' flat views

    sT = nc.dram_tensor("sT", [64, b_shard], F32R, kind="ExternalInput")
    a0T = nc.dram_tensor("a0T", [16, b_shard], F32R, kind="ExternalInput")
    a04d = nc.dram_tensor("a04d", [128, ngroups * TILE], F32R, kind="ExternalInput")
    rad4d = nc.dram_tensor(
        "rad4d", [N_STEPS, ngroups, 128, TILE], F32R, kind="ExternalInput"
    )
    W1d = nc.dram_tensor("W1d", [80, 128], F32R, kind="ExternalInput")
    W1a4d = nc.dram_tensor("W1a4d", [128, 128], F32R, kind="ExternalInput")
    W2d = nc.dram_tensor("W2d", [128, 128], F32R, kind="ExternalInput")
    # 2*GRP lhsT variants: [2j] puts W3v at cols 32j..32j+15 (dv lane),
    # [2j+1] at cols 32j+16..32j+31 (v lane) -- all accumulate into one bank
    W3jd = nc.dram_tensor("W3jd", [2 * GRP, 128, 128], F32R, kind="ExternalInput")
    b1td = nc.dram_tensor("b1td", [128, N_STEPS], F32, kind="ExternalInput")
    b2d = nc.dram_tensor("b2d", [128, 1], F32, kind="ExternalInput")
    b3v4d = nc.dram_tensor("b3v4d", [128, 1], F32, kind="ExternalInput")
    redd = nc.dram_tensor("redd", [128, 8], F32R, kind="ExternalInput")
    fbd = nc.dram_tensor("fbd", [GRP, 1], F32, kind="ExternalInput")
    outd = nc.dram_tensor("out", [ntiles, TILE], F32, kind="ExternalOutput")

    with tile.TileContext(nc) as tc:
        with (
            tc.tile_pool(name="const", bufs=1) as cpool,
            tc.tile_pool(name="work", bufs=2) as wpool,
            tc.tile_pool(name="rstream", bufs=2) as rpool,
            tc.tile_pool(name="ps", bufs=1, space=bass.MemorySpace.PSUM) as psum,
        ):
            W1s = cpool.tile([80, 128], F32R)
            nc.sync.dma_start(W1s[:], W1d[:])
            W1a4s = cpool.tile([128, 128], F32R)
            nc.sync.dma_start(W1a4s[:], W1a4d[:])
            W2s = cpool.tile([128, 128], F32R)
            nc.sync.dma_start(W2s[:], W2d[:])
            W3js = cpool.tile([128, 2 * GRP * 128], F32R)
            for j in range(2 * GRP):
                nc.sync.dma_start(W3js[:, j * 128 : (j + 1) * 128], W3jd[j])
            b1ts = cpool.tile([128, N_STEPS], F32)
            nc.sync.dma_start(b1ts[:], b1td[:])
            b2s = cpool.tile([128, 1], F32)
            nc.sync.dma_start(b2s[:], b2d[:])
            b3v4s = cpool.tile([128, 1], F32)
            nc.sync.dma_start(b3v4s[:], b3v4d[:])
            reds = cpool.tile([128, 8], F32R)
            nc.sync.dma_start(reds[:], redd[:])
            fbias = cpool.tile([GRP, 1], F32)
            nc.sync.dma_start(fbias[:], fbd[:])

            xT = cpool.tile([80, b_shard], F32R)
            nc.sync.dma_start(xT[0:16, :], a0T[:])
            nc.sync.dma_start(xT[16:80, :], sT[:])
            a4 = cpool.tile([128, ngroups * TILE], F32R)
            nc.sync.dma_start(a4[:], a04d[:])
            accs = [
                cpool.tile([128, TILE], F32, name=f"acc4_{g}", tag=f"acc4_{g}")
                for g in range(ngroups)
            ]
            fout = cpool.tile([GRP, b_shard // GRP], F32)

            # view of xT's a-rows as [feat, group, j, n]
            xTa = xT[0:16, :].rearrange("p (g j n) -> p g j n", j=GRP, n=TILE)

            for s in range(N_STEPS):
                b1c = b1ts[:, s : s + 1]
                for g in range(ngroups):
                    rad4g = rpool.tile([128, TILE], F32R, tag="rad4g")
                    nc.sync.dma_start(rad4g[:], rad4d[s, g])
                    v4 = psum.tile([128, TILE], F32, tag="v4")
                    dv4 = psum.tile([128, TILE], F32, tag="dv4")
                    for j in range(GRP):
                        t = g * GRP + j
                        cols = slice(t * TILE, (t + 1) * TILE)
                        pre1 = psum.tile([128, TILE], F32, tag="pre1", bufs=2)
                        dpre1 = psum.tile([128, TILE], F32, tag="dpre1", bufs=2)
                        nc.tensor.matmul(
                            pre1[:], W1s[:], xT[:, cols], start=True, stop=True
                        )
                        nc.tensor.matmul(
                            dpre1[:],
                            W1a4s[32 * j : 32 * j + 16, :],
                            rad4g[32 * j : 32 * j + 16, :],
                            start=True,
                            stop=True,
                            tile_position=(32 * j, 0),
                        )
                        h1 = wpool.tile([128, TILE], F32R, tag="h1")
                        if not no_relu:
                            nc.scalar.activation(h1[:], pre1[:], AF.Relu, bias=b1c)
                        dh1 = wpool.tile([128, TILE], F32R, tag="dh1")
                        if not no_msel:
                            nc.vector.scalar_tensor_tensor(
                                dh1[:], h1[:], 0.0, dpre1[:], OP.is_gt, OP.mult
                            )
                        pre2 = psum.tile([128, TILE], F32, tag="pre2")
                        dpre2 = psum.tile([128, TILE], F32, tag="dpre2")
                        nc.tensor.matmul(
                            pre2[:], W2s[:], h1[:], start=True, stop=True
                        )
                        nc.tensor.matmul(
                            dpre2[:], W2s[:], dh1[:], start=True, stop=True
                        )
                        h2 = wpool.tile([128, TILE], F32R, tag="h2")
                        if not no_relu:
                            if has_b2:
                                nc.scalar.activation(
                                    h2[:], pre2[:], AF.Relu, bias=b2s[:, 0:1]
                                )
                            else:
                                nc.scalar.activation(h2[:], pre2[:], AF.Relu)
                        dh2 = wpool.tile([128, TILE], F32R, tag="dh2")
                        if not no_msel:
                            nc.vector.scalar_tensor_tensor(
                                dh2[:], h2[:], 0.0, dpre2[:], OP.is_gt, OP.mult
                            )
                        if not no_l3:
                            w3j = W3js[:, j * 128 : (j + 1) * 128]
                            nc.tensor.matmul(
                                v4[:], w3j, h2[:],
                                start=(j == 0), stop=(j == GRP - 1),
                                skip_group_check=True,
                            )
                            nc.tensor.matmul(
                                dv4[:], w3j, dh2[:],
                                start=(j == 0), stop=(j == GRP - 1),
                                skip_group_check=True,
                            )
                    # folded group ops at full 128-lane width
                    gcols = slice(g * TILE, (g + 1) * TILE)
                    nc.vector.scalar_tensor_tensor(
                        a4[:, gcols], v4[:], DELTA, a4[:, gcols], OP.mult, OP.add
                    )
                    if has_b3v:
                        nc.vector.tensor_scalar_add(
                            a4[:, gcols], a4[:, gcols], b3v4s[:, 0:1]
                        )
                    dv4s = wpool.tile([128, TILE], F32, tag="dv4s")
                    nc.scalar.activation(dv4s[:], dv4[:], AF.Copy)
                    prod4 = wpool.tile([128, TILE], F32, tag="prod4")
                    eng = nc.vector if acc_dve else nc.gpsimd
                    eng.tensor_mul(prod4[:], dv4s[:], rad4g[:].bitcast(F32))
                    if s == 0:
                        eng.tensor_copy(accs[g][:], prod4[:])
                    else:
                        eng.tensor_add(accs[g][:], accs[g][:], prod4[:])
                    # scatter this group's folded a back to flat xT a-rows
                    if s != N_STEPS - 1 and not no_adma:
                        for j in range(GRP):
                            nc.sync.dma_start(
                                xTa[:, g, j, :],
                                a4[32 * j : 32 * j + 16, gcols],
                            )

            for g in range(ngroups):
                gcols = slice(g * TILE, (g + 1) * TILE)
                z4 = wpool.tile([128, TILE], F32R, tag="z4")
                nc.vector.tensor_mul(z4[:], a4[:, gcols], a4[:, gcols])
                acc4r = wpool.tile([128, TILE], F32R, tag="acc4r")
                nc.vector.tensor_copy(acc4r[:], accs[g][:])
                fin = psum.tile([GRP, TILE], F32, tag="pre2")
                nc.tensor.matmul(
                    fin[:], reds[:, 0:GRP], z4[:], start=True, stop=False
                )
                nc.tensor.matmul(
                    fin[:], reds[:, GRP : 2 * GRP], acc4r[:], start=False, stop=True
                )
                fo = fout[:, g * TILE : (g + 1) * TILE]
                nc.scalar.activation(fo, fin[:], AF.Identity, bias=fbias[:, 0:1])
                nc.sync.dma_start(
                    outd[g * GRP : (g + 1) * GRP, :],
                    fo,
                )
    nc.finalize()
    return nc


def make_in_maps(s, a0, rad, W1, b1, W2, b2, W3, b3, n_cores=N_CORES):
    """Host-side prep: shard along batch, transpose to feature-major, fold
    the t column / biases, pre-scale rad by -DELTA, build folded layouts."""
    B = s.shape[0]
    b_shard = B // n_cores
    ntiles = b_shard // TILE
    ngroups = ntiles // GRP
    s = np.asarray(s, np.float32)
    a0 = np.asarray(a0, np.float32)
    W1 = np.asarray(W1, np.float32)
    b1 = np.asarray(b1, np.float32)
    W2 = np.asarray(W2, np.float32)
    b2 = np.asarray(b2, np.float32)
    W3 = np.asarray(W3, np.float32)
    b3 = np.asarray(b3, np.float32)

    t_mids = (np.arange(N_STEPS, dtype=np.float64) + 0.5) / N_STEPS
    t_mids = t_mids.astype(np.float32)
    b1t = (b1[None, :] + t_mids[:, None] * W1[80][None, :]).astype(np.float32).T
    b1t = np.ascontiguousarray(b1t)
    W1_80 = np.ascontiguousarray(W1[:80])
    W1a4 = np.zeros((128, 128), np.float32)
    for j in range(GRP):
        W1a4[32 * j : 32 * j + 16] = W1[:16]
    W3v = (W3[:, :16] + W3[:, 16:]).astype(np.float32)  # [128, 16]
    W3j = np.zeros((GRP, 128, 128), np.float32)
    for j in range(GRP):
        W3j[j][:, 32 * j : 32 * j + 16] = W3v
    b3v = (b3[:16] + b3[16:]).astype(np.float32)
    b2c = np.ascontiguousarray(b2[:, None])
    b3v4 = np.zeros((128, 1), np.float32)
    for j in range(GRP):
        b3v4[32 * j : 32 * j + 16, 0] = DELTA * b3v
    red = np.zeros((128, 8), np.float32)
    for j in range(GRP):
        red[32 * j : 32 * j + 16, j] = -0.5
        red[32 * j : 32 * j + 16, GRP + j] = -1.0 / DELTA
    fb = np.full((GRP, 1), -8.0 * LOG_2PI, np.float32)

    has_b2 = bool(np.any(b2 != 0))
    has_b3v = bool(np.any(b3v != 0))

    in_maps = []
    for i in range(n_cores):
        sl = slice(i * b_shard, (i + 1) * b_shard)
        radf = (-DELTA) * (
            2.0 * np.asarray(rad[:, sl, :], np.float32) - 1.0
        )  # [32, b, 16]
        # folded rad: [32, g, 32j+f, n]
        r5 = radf.reshape(N_STEPS, ngroups, GRP, TILE, 16)
        rad4 = np.zeros((N_STEPS, ngroups, GRP, 32, TILE), np.float32)
        rad4[:, :, :, :16, :] = r5.transpose(0, 1, 2, 4, 3)
        rad4 = rad4.reshape(N_STEPS, ngroups, 128, TILE)
        # folded a0: [32j+f, g*TILE + n]
        a5 = a0[sl].reshape(ngroups, GRP, TILE, 16)
        a04 = np.zeros((GRP, 32, ngroups, TILE), np.float32)
        a04[:, :16, :, :] = a5.transpose(1, 3, 0, 2)
        a04 = a04.reshape(128, ngroups * TILE)
        in_maps.append(
            {
                "sT": np.ascontiguousarray(s[sl].T),
                "a0T": np.ascontiguousarray(a0[sl].T),
                "a04d": np.ascontiguousarray(a04),
                "rad4d": np.ascontiguousarray(rad4),
                "W1d": W1_80,
                "W1a4d": W1a4,
                "W2d": W2,
                "W3jd": W3j,
                "b1td": b1t,
                "b2d": b2c,
                "b3v4d": b3v4,
                "redd": red,
                "fbd": fb,
            }
        )
    return in_maps, has_b2, has_b3v, b_shard


def kernel(**inputs) -> np.ndarray:
    global last_exec_time_ns, last_results
    in_maps, has_b2, has_b3v, b_shard = make_in_maps(**inputs)
    nc = build_kernel(b_shard, has_b2, has_b3v)
    trace = bool(os.environ.get("KERNEL_TRACE"))
    res = run_bass_kernel_spmd(
        nc, in_maps, core_ids=list(range(N_CORES)), trace=trace
    )
    last_exec_time_ns = res.exec_time_ns
    last_results = res
    out = np.concatenate([r["out"].reshape(-1) for r in res.results])
    return out.astype(np.float32)
